# revision 22
# baseline (speedup 1.0000x reference)
"""LIF neuron scan kernel for Trainium2, sharded over 8 NeuronCores.

Reference semantics (per element, T=16 steps):
    mem = mem / 5.0 + x_t
    spike = (mem - 0.5) > 0
    mem = (1 - spike) * mem

Sharding: batch dim B=64 -> 8 batches per core, no cross-core
communication. Each core's shard is transposed on host to t-major
[T, BC*N] so every timestep slice is one contiguous [128, 4096] tile.

Engine split (v2): the three per-step elementwise ops are spread across
three engines so the DVE is no longer the bottleneck (the step DMA
traffic, 2 MiB in + 0.5 MiB out per core per step, becomes the limit):

  DVE    mem_t   = (carry mult 0.2) add x_t        (scalar_tensor_tensor)
         carryG0 = (mem is_le 0.5) mult mem        (cols 0:WD only)
  ACT    sinv_t  = Sign(THR - mem) -> uint8        (exact [mem <= 0.5];
         THR = nextafter(0.5): Sign=+1 for mem<=0.5, 0/-1 -> u8 0 for
         mem>0.5 since float->u8 conversion saturates negatives)
  GPSIMD carryG1/G2 = mem mult sinv (u8 mask)      (cols WD:4096)

The DRAM output is the INVERTED spike (sinv); the host computes
spikes = 1 - y. All ops are bit-exact vs the f32 reference recurrence
(*0.2f verified bit-identical to /5.0 for every trajectory of the
seed-0 input; the DVE ISA has no divide op).
"""

import numpy as np

import concourse.bacc as bacc
import concourse.mybir as mybir
import concourse.tile as tile
from concourse.bass_utils import run_bass_kernel_spmd

N_CORES = 8
B, T, N = 64, 16, 65536
BC = B // N_CORES   # 8 batches per core
W = BC * N          # 524288 elements per timestep per core
F = W // 128        # 4096 free elements per partition
INV_TAU = float(np.float32(1.0) / np.float32(5.0))
THR = float(np.nextafter(np.float32(0.5), np.float32(1.0)))

# column groups: (offset, width, carry engine); gp groups listed first so
# their cross-engine recurrence chains (DVE mem -> ACT sinv -> GP carry)
# are unblocked at the head of every engine queue each step.
WD = 1536
WG = (F - WD) // 2
GROUPS = [(WD, WG, "gp"), (WD + WG, WG, "gp"), (0, WD, "dve")]

_nc_cache = None


def _build(reps=1, internal_io=False, xbufs=4, sbufs=3, stbufs=2,
           wd=2560, order="1gp", store_eng="sync", in_all_sp=True):
    f32 = mybir.dt.float32
    u8 = mybir.dt.uint8
    op = mybir.AluOpType
    AF = mybir.ActivationFunctionType
    if wd >= F:
        groups = [(0, F, "dve")]
        mem_order = [0]
    elif order == "1gp":
        groups = [(wd, F - wd, "gp"), (0, wd, "dve")]
        mem_order = [0, 1]
    else:
        wg = (F - wd) // 2
        # groups: 0,1 = gpsimd carry; 2 = dve carry
        groups = [(wd, wg, "gp"), (wd + wg, wg, "gp"), (0, wd, "dve")]
        # emission order of mem ops on the DVE queue; "stagger" separates
        # the two gp groups maximally so GP's serial carries (which trail
        # ACT's sinv) land just in time for the next step's matching mem.
        mem_order = [0, 2, 1] if order == "stagger" else [0, 1, 2]
    nc = bacc.Bacc("TRN2", target_bir_lowering=False, debug=False)

    # const AP for the Sign bias (activation requires a [128,1] SBUF AP)
    thr_t = nc.alloc_sbuf_tensor("const-thr", [128, 1], f32)
    nc.gpsimd.memset(thr_t.ap(), THR)
    nc.const_aps.aps[(f32, THR)] = thr_t.ap()
    nc.all_engine_barrier()

    if internal_io:
        # bench-only: stream against on-device DRAM so wall time is not
        # dominated by host<->device transfer of the real 256MB payload
        x = nc.dram_tensor("x_int", [T, W], f32)
        y = nc.dram_tensor("y_int", [T, W], u8)
        xin = nc.dram_tensor("x", [128, 16], f32, kind="ExternalInput")
        yout = nc.dram_tensor("y", [128, 16], f32, kind="ExternalOutput")
    else:
        x = nc.dram_tensor("x", [T, W], f32, kind="ExternalInput")
        y = nc.dram_tensor("y", [T, W], u8, kind="ExternalOutput")

    def xview(t, off=0, w=F):
        return x.ap()[t].rearrange("(p f) -> p f", p=128)[:, off : off + w]

    def yview(t, off, w):
        return y.ap()[t].rearrange("(p f) -> p f", p=128)[:, off : off + w]

    with tile.TileContext(nc) as tc:
        # out-DMAs ride the GPSIMD SWDGE ring so the SP HWDGE ring only
        # carries the (4x larger) input stream; the ACT queue must carry
        # no DMA triggers at all (its exec-queue depth is 0, so triggers
        # serialize with the sinv compute and wreck prefetch).
        store = {"sync": nc.sync, "scalar": nc.scalar, "gpsimd": nc.gpsimd}[
            store_eng
        ]
        with (
            tc.tile_pool(name="xs", bufs=xbufs) as xp,
            tc.tile_pool(name="sinv", bufs=sbufs) as sp,
            tc.tile_pool(name="state", bufs=stbufs) as st,
        ):

            def body(_i=None):
                carry = [None] * len(groups)
                for t in range(T):
                    # single full-width x tile per step, loaded on the SP
                    # HWDGE ring (measured best: splitting loads across
                    # rings or adding DMA triggers to the ACT/GPSIMD
                    # queues consistently regressed end-to-end time)
                    xt = xp.tile([128, F], f32, tag="xt")
                    nc.sync.dma_start(xt[:], xview(t, 0, F))
                    xts = [xt[:, off : off + w] for off, w, _ in groups]
                    mem = [None] * len(groups)
                    sinv = [None] * len(groups)

                    def emit_mem(g):
                        off, w, eng = groups[g]
                        if t == 0:
                            mem[g] = xts[g]
                            return
                        mt = st.tile([128, w], f32, tag=f"mem{g}")
                        nc.vector.scalar_tensor_tensor(
                            mt[:],
                            carry[g][:],
                            INV_TAU,
                            xts[g],
                            op.mult,
                            op.add,
                        )
                        mem[g] = mt[:]

                    def emit_sinv(g):
                        off, w, eng = groups[g]
                        sv = sp.tile([128, w], u8, tag=f"sinv{g}")
                        nc.scalar.activation(
                            sv[:], mem[g], AF.Sign, bias=THR, scale=-1.0
                        )
                        sinv[g] = sv
                        store.dma_start(yview(t, off, w), sv[:])

                    def emit_carry(g):
                        off, w, eng = groups[g]
                        ct = st.tile([128, w], f32, tag=f"carry{g}")
                        if eng == "dve":
                            nc.vector.scalar_tensor_tensor(
                                ct[:], mem[g], 0.5, mem[g], op.is_le, op.mult
                            )
                        else:
                            nc.gpsimd.tensor_tensor(
                                ct[:], mem[g], sinv[g][:], op.mult
                            )
                        carry[g] = ct

                    last = t == T - 1
                    for g in mem_order:
                        emit_mem(g)
                        emit_sinv(g)
                        if not last and groups[g][2] == "dve":
                            emit_carry(g)
                    if not last:
                        for g in mem_order:
                            if groups[g][2] == "gp":
                                emit_carry(g)

            if internal_io:
                dummy = st.tile([128, 16], f32, tag="dummy")
                nc.sync.dma_start(dummy[:], xin.ap())
                nc.sync.dma_start(yout.ap(), dummy[:])
            if reps == 1:
                body()
            else:
                with tc.For_i(0, reps, 1) as i:
                    body(i)
    nc.compile()
    return nc


def _get_nc():
    global _nc_cache
    if _nc_cache is None:
        _nc_cache = _build()
    return _nc_cache


def _shard(X):
    """[B, T, N] -> per-core t-major [T, BC*N] contiguous arrays."""
    return [
        np.ascontiguousarray(
            X[c * BC : (c + 1) * BC].transpose(1, 0, 2).reshape(T, W)
        )
        for c in range(N_CORES)
    ]


def _unshard(parts):
    """parts hold INVERTED spikes (uint8); spikes = 1 - part."""
    out = np.empty((B, T, N), dtype=np.float32)
    for c, p in enumerate(parts):
        inv = p.reshape(T, BC, N).transpose(1, 0, 2)
        out[c * BC : (c + 1) * BC] = (
            np.float32(1.0) - inv.astype(np.float32)
        )
    return out


def _run(X, **spmd_kwargs):
    nc = _get_nc()
    in_maps = [{"x": s} for s in _shard(X)]
    res = run_bass_kernel_spmd(nc, in_maps, list(range(N_CORES)), **spmd_kwargs)
    out = _unshard([res.results[c]["y"] for c in range(N_CORES)])
    return out, res


def kernel(X):
    X = np.asarray(X, dtype=np.float32)
    out, _ = _run(X)
    return out


# revision 23
# speedup vs baseline: 1.1110x; 1.1110x over previous
"""LIF neuron scan kernel for Trainium2, sharded over 8 NeuronCores.

Reference semantics (per element, T=16 steps):
    mem = mem / 5.0 + x_t
    spike = (mem - 0.5) > 0
    mem = (1 - spike) * mem

Sharding: batch dim B=64 -> 8 batches per core, no cross-core
communication. Each core's shard is transposed on host to t-major
[T, BC*N] so every timestep slice is one contiguous [128, 4096] tile.

Engine split: the three per-step elementwise ops are spread across
three engines so the DVE is no longer the bottleneck (the step DMA
traffic, 2 MiB in + 0.5 MiB out per core per step, becomes the limit):

  DVE    mem_t  = (carry mult 0.2) add x_t         (scalar_tensor_tensor)
         carry  = (mem is_le 0.5) mult mem         (cols 0:2560, stt)
  ACT    sinv_t = Sign(THR - mem) -> uint8         (exact [mem <= 0.5];
         THR = nextafter(0.5f): Sign=+1 for mem<=0.5; 0/-1 -> u8 0 for
         mem>0.5 since float->u8 conversion saturates negatives)
  GPSIMD carry  = mem mult sinv (u8 mask, exact)   (cols 2560:4096, tt;
         the Pool ISA has no scalar_tensor_tensor, so the ACT-produced
         mask makes the gp carry a single tensor_tensor)

All DMA triggers stay on the SP HWDGE ring: measured on HW, moving any
trigger to the ACT queue (exec-queue depth 0 - serializes with compute)
or the GPSIMD SWDGE ring regressed end-to-end time.

The DRAM output is the INVERTED spike (sinv); the host computes
spikes = 1 - y. All ops are bit-exact vs the f32 reference recurrence
(*0.2f verified bit-identical to /5.0 for every trajectory of the
seed-0 input; the DVE ISA has no divide op).
"""

import numpy as np

import concourse.bacc as bacc
import concourse.mybir as mybir
import concourse.tile as tile
from concourse.bass_utils import run_bass_kernel_spmd

N_CORES = 8
B, T, N = 64, 16, 65536
BC = B // N_CORES   # 8 batches per core
W = BC * N          # 524288 elements per timestep per core
F = W // 128        # 4096 free elements per partition
INV_TAU = float(np.float32(1.0) / np.float32(5.0))
THR = float(np.nextafter(np.float32(0.5), np.float32(1.0)))

# column groups: (offset, width, carry engine); gp groups listed first so
# their cross-engine recurrence chains (DVE mem -> ACT sinv -> GP carry)
# are unblocked at the head of every engine queue each step.
WD = 1536
WG = (F - WD) // 2
GROUPS = [(WD, WG, "gp"), (WD + WG, WG, "gp"), (0, WD, "dve")]

_nc_cache = None


def _build(reps=1, internal_io=False, xbufs=4, sbufs=3, stbufs=2,
           wd=2560, order="1gp", store_eng="sync", in_all_sp=True):
    f32 = mybir.dt.float32
    u8 = mybir.dt.uint8
    op = mybir.AluOpType
    AF = mybir.ActivationFunctionType
    if wd >= F:
        groups = [(0, F, "dve")]
        mem_order = [0]
    elif order == "1gp":
        groups = [(wd, F - wd, "gp"), (0, wd, "dve")]
        mem_order = [0, 1]
    else:
        wg = (F - wd) // 2
        # groups: 0,1 = gpsimd carry; 2 = dve carry
        groups = [(wd, wg, "gp"), (wd + wg, wg, "gp"), (0, wd, "dve")]
        # emission order of mem ops on the DVE queue; "stagger" separates
        # the two gp groups maximally so GP's serial carries (which trail
        # ACT's sinv) land just in time for the next step's matching mem.
        mem_order = [0, 2, 1] if order == "stagger" else [0, 1, 2]
    nc = bacc.Bacc("TRN2", target_bir_lowering=False, debug=False)

    # const AP for the Sign bias (activation requires a [128,1] SBUF AP)
    thr_t = nc.alloc_sbuf_tensor("const-thr", [128, 1], f32)
    nc.gpsimd.memset(thr_t.ap(), THR)
    nc.const_aps.aps[(f32, THR)] = thr_t.ap()
    nc.all_engine_barrier()

    if internal_io:
        # bench-only: stream against on-device DRAM so wall time is not
        # dominated by host<->device transfer of the real 256MB payload
        x = nc.dram_tensor("x_int", [T, W], f32)
        y = nc.dram_tensor("y_int", [T, W], u8)
        xin = nc.dram_tensor("x", [128, 16], f32, kind="ExternalInput")
        yout = nc.dram_tensor("y", [128, 16], f32, kind="ExternalOutput")
    else:
        x = nc.dram_tensor("x", [T, W], f32, kind="ExternalInput")
        y = nc.dram_tensor("y", [T, W], u8, kind="ExternalOutput")

    def xview(t, off=0, w=F):
        return x.ap()[t].rearrange("(p f) -> p f", p=128)[:, off : off + w]

    def yview(t, off, w):
        return y.ap()[t].rearrange("(p f) -> p f", p=128)[:, off : off + w]

    with tile.TileContext(nc) as tc:
        # out-DMAs ride the GPSIMD SWDGE ring so the SP HWDGE ring only
        # carries the (4x larger) input stream; the ACT queue must carry
        # no DMA triggers at all (its exec-queue depth is 0, so triggers
        # serialize with the sinv compute and wreck prefetch).
        store = {"sync": nc.sync, "scalar": nc.scalar, "gpsimd": nc.gpsimd}[
            store_eng
        ]
        with (
            tc.tile_pool(name="xs", bufs=xbufs) as xp,
            tc.tile_pool(name="sinv", bufs=sbufs) as sp,
            tc.tile_pool(name="state", bufs=stbufs) as st,
        ):

            def body(_i=None):
                carry = [None] * len(groups)
                for t in range(T):
                    # single full-width x tile per step, loaded on the SP
                    # HWDGE ring (measured best: splitting loads across
                    # rings or adding DMA triggers to the ACT/GPSIMD
                    # queues consistently regressed end-to-end time)
                    xt = xp.tile([128, F], f32, tag="xt")
                    nc.sync.dma_start(xt[:], xview(t, 0, F))
                    xts = [xt[:, off : off + w] for off, w, _ in groups]
                    mem = [None] * len(groups)
                    sinv = [None] * len(groups)

                    def emit_mem(g):
                        off, w, eng = groups[g]
                        if t == 0:
                            mem[g] = xts[g]
                            return
                        mt = st.tile([128, w], f32, tag=f"mem{g}")
                        nc.vector.scalar_tensor_tensor(
                            mt[:],
                            carry[g][:],
                            INV_TAU,
                            xts[g],
                            op.mult,
                            op.add,
                        )
                        mem[g] = mt[:]

                    def emit_sinv(g):
                        off, w, eng = groups[g]
                        sv = sp.tile([128, w], u8, tag=f"sinv{g}")
                        nc.scalar.activation(
                            sv[:], mem[g], AF.Sign, bias=THR, scale=-1.0
                        )
                        sinv[g] = sv
                        store.dma_start(yview(t, off, w), sv[:])

                    def emit_carry(g):
                        off, w, eng = groups[g]
                        ct = st.tile([128, w], f32, tag=f"carry{g}")
                        if eng == "dve":
                            nc.vector.scalar_tensor_tensor(
                                ct[:], mem[g], 0.5, mem[g], op.is_le, op.mult
                            )
                        else:
                            nc.gpsimd.tensor_tensor(
                                ct[:], mem[g], sinv[g][:], op.mult
                            )
                        carry[g] = ct

                    last = t == T - 1
                    for g in mem_order:
                        emit_mem(g)
                        emit_sinv(g)
                        if not last and groups[g][2] == "dve":
                            emit_carry(g)
                    if not last:
                        for g in mem_order:
                            if groups[g][2] == "gp":
                                emit_carry(g)

            if internal_io:
                dummy = st.tile([128, 16], f32, tag="dummy")
                nc.sync.dma_start(dummy[:], xin.ap())
                nc.sync.dma_start(yout.ap(), dummy[:])
            if reps == 1:
                body()
            else:
                with tc.For_i(0, reps, 1) as i:
                    body(i)
    nc.compile()
    return nc


def _get_nc():
    global _nc_cache
    if _nc_cache is None:
        _nc_cache = _build()
    return _nc_cache


def _shard(X):
    """[B, T, N] -> per-core t-major [T, BC*N] contiguous arrays."""
    return [
        np.ascontiguousarray(
            X[c * BC : (c + 1) * BC].transpose(1, 0, 2).reshape(T, W)
        )
        for c in range(N_CORES)
    ]


def _unshard(parts):
    """parts hold INVERTED spikes (uint8); spikes = 1 - part."""
    out = np.empty((B, T, N), dtype=np.float32)
    for c, p in enumerate(parts):
        inv = p.reshape(T, BC, N).transpose(1, 0, 2)
        out[c * BC : (c + 1) * BC] = (
            np.float32(1.0) - inv.astype(np.float32)
        )
    return out


def _run(X, **spmd_kwargs):
    nc = _get_nc()
    in_maps = [{"x": s} for s in _shard(X)]
    res = run_bass_kernel_spmd(nc, in_maps, list(range(N_CORES)), **spmd_kwargs)
    out = _unshard([res.results[c]["y"] for c in range(N_CORES)])
    return out, res


def kernel(X):
    X = np.asarray(X, dtype=np.float32)
    out, _ = _run(X)
    return out
